# revision 12
# baseline (speedup 1.0000x reference)
"""Trainium2 Bass kernel for nn_CorefModel (topk_masking).

8 cores, data-parallel over the K=768 top spans (96 rows/core). One SPMD
program; everything core-specific arrives as input tensors.

Per core:
  A) span embeddings replicated: start/end indirect row gathers, banded
     masked-softmax head attention in fp32 (exploits sorted starts and
     span width <= 30), PE transposes -> spanT [2304, 768] fp32 + bf16/fp32
     DRAM gather tables.
  B) fp32 coarse scores for its 96 rows + host-precomputed additive bias
     (mention-score pair sums + distance prior + antecedent mask), then
     top-56/row via DVE max8/max_index/match_replace.
  C) dma_gather(transpose=True) of selected span/A columns in bf16, FFNN via
     PE matmuls (sim term + identity-add A term + one-hot distance term)
     accumulated in PSUM, T term added on DVE, ACT relu(+b1), w2 matmul.
  D) out[k, :] = [0, slow + top_fast + b2] with -inf forcing for rows k < 50.
"""

import sys

sys.path.insert(0, "/opt/trn_rl_repo")
from contextlib import ExitStack

import numpy as np
import ml_dtypes

import concourse.bass as bass
import concourse.bacc as bacc
import concourse.mybir as mybir
from concourse.masks import make_identity
from concourse.tile import TileContext
from concourse.bass_utils import run_bass_kernel_spmd

F32 = mybir.dt.float32
BF16 = mybir.dt.bfloat16
I32 = mybir.dt.int32
U16 = mybir.dt.uint16
I16 = mybir.dt.int16
BF = ml_dtypes.bfloat16

W = 4096
D = 768
K = 768
NCORES = 8
KPC = K // NCORES          # 96
SPAN = 3 * D               # 2304
HCH = 8
HP = HCH * 128             # 1024
H = 1000
F = 20
CP = 56                    # padded antecedents (7 max8 rounds)
C = 50
NPAIR = KPC * CP           # 5376
NBLK = 6
BLK = NPAIR // NBLK        # 896
KGRP = BLK // CP           # 16
NSUB = BLK // 128          # 7
WCH = W // 128             # 32
DCH = D // 128             # 6
SCH = SPAN // 128          # 18
NEG = -1.0e30

AX = mybir.AxisListType.X
OP = mybir.AluOpType
AF = mybir.ActivationFunctionType


def _bucket_np(d):
    d = np.asarray(d)
    logspace = np.floor(
        np.log2(np.maximum(d, 1).astype(np.float64))
    ).astype(np.int64) + 3
    return np.clip(np.where(d <= 4, d, logspace), 0, 9)


def _split512(lo, hi):
    """Split [lo, hi) at 512-column PSUM bank boundaries."""
    segs = []
    while lo < hi:
        nxt = min(hi, ((lo // 512) + 1) * 512)
        segs.append((lo, nxt))
        lo = nxt
    return segs


def build_program(starts_np):
    nc = bacc.Bacc(None, target_bir_lowering=False, num_swdge_queues=4)

    md = nc.declare_dram_parameter("md", [W, D], F32, isOutput=False)
    md1 = nc.declare_dram_parameter("md1", [W, D], F32, isOutput=False)
    starts_p = nc.declare_dram_parameter("starts", [K], I32, isOutput=False)
    ends_p = nc.declare_dram_parameter("ends", [K], I32, isOutput=False)
    startsf_p = nc.declare_dram_parameter("startsf", [1, K], F32, isOutput=False)
    endsf_p = nc.declare_dram_parameter("endsf", [1, K], F32, isOutput=False)
    awh_p = nc.declare_dram_parameter("awh", [1, D], F32, isOutput=False)
    abv_p = nc.declare_dram_parameter("abv", [128, 1], F32, isOutput=False)
    cw_p = nc.declare_dram_parameter("cw", [SPAN, SPAN], F32, isOutput=False)
    cbt_p = nc.declare_dram_parameter("cbt", [128, SCH], F32, isOutput=False)
    w1t_p = nc.declare_dram_parameter("w1t", [SPAN, HP], BF16, isOutput=False)
    w1a_p = nc.declare_dram_parameter("w1a", [SPAN, HP], BF16, isOutput=False)
    w1s_p = nc.declare_dram_parameter("w1s", [SPAN, HP], BF16, isOutput=False)
    d10_p = nc.declare_dram_parameter("d10", [16, HP], BF16, isOutput=False)
    b1t_p = nc.declare_dram_parameter("b1t", [128, HCH], F32, isOutput=False)
    w2t_p = nc.declare_dram_parameter("w2t", [128, HCH], BF16, isOutput=False)
    b2v_p = nc.declare_dram_parameter("b2v", [KPC, 1], F32, isOutput=False)
    biasmx_p = nc.declare_dram_parameter("biasmx", [KPC, K], F32, isOutput=False)
    forcem_p = nc.declare_dram_parameter("forcem", [KPC, 51], F32, isOutput=False)
    myidx_p = nc.declare_dram_parameter("myidx", [128, 1], I32, isOutput=False)
    kvec_p = nc.declare_dram_parameter("kvec", [KPC, 1], F32, isOutput=False)
    out_p = nc.declare_dram_parameter("out", [KPC, 51], F32, isOutput=True)

    span_tab = nc.dram_tensor("span_tab", [K, SPAN], BF16)
    a_tab = nc.dram_tensor("a_tab", [K, HP], BF16)
    span_f32 = nc.dram_tensor("span_f32", [K, SPAN], F32)
    hyb_d = nc.dram_tensor("hyb_d", [W, D], F32)
    idxflat_d = nc.dram_tensor("idxflat_d", [NPAIR], U16)
    bucket_d = nc.dram_tensor("bucket_d", [NPAIR], F32)
    slow_d = nc.dram_tensor("slow_d", [NPAIR], F32)

    s_np = np.asarray(starts_np).astype(np.int64)
    assert np.all(np.diff(s_np) >= 0)
    plo = [int(np.searchsorted(s_np, 128 * t, side="left")) for t in range(WCH + 1)]
    bandlo = [plo[max(t - 1, 0)] for t in range(WCH)]
    bandhi = [plo[t + 1] for t in range(WCH)]
    bw = [bandhi[t] - bandlo[t] for t in range(WCH)]
    wtoff = np.concatenate([[0], np.cumsum(bw)]).astype(int)
    MAXBW = max(max(bw), 1)

    def banded_matmuls(t, out_ps, lhsT, wT):
        """Primary/secondary banded matmuls into out_ps[:, span-col ranges]."""
        lo = bandlo[t]
        plo_t, phi_t = plo[t], plo[t + 1]
        if phi_t > plo_t:  # primary: spans starting in chunk t
            for a, b in _split512(plo_t, phi_t):
                o = int(wtoff[t]) + (a - lo)
                nc.tensor.matmul(
                    out=out_ps[:, a:b], lhsT=lhsT, rhs=wT[:, o : o + (b - a)],
                    start=False, stop=False, skip_group_check=True,
                )
        if t > 0 and plo[t] > plo[t - 1]:  # secondary: prev-chunk spans
            for a, b in _split512(plo[t - 1], plo[t]):
                o = int(wtoff[t]) + (a - lo)
                nc.tensor.matmul(
                    out=out_ps[:, a:b], lhsT=lhsT, rhs=wT[:, o : o + (b - a)],
                    start=False, stop=False, skip_group_check=True,
                )

    with TileContext(nc) as tc, ExitStack() as top:
        const_pool = top.enter_context(tc.tile_pool(name="const", bufs=1))
        myTb_pool = top.enter_context(tc.tile_pool(name="myTb", bufs=1))
        TT_pool = top.enter_context(tc.tile_pool(name="TTp", bufs=1))
        tk_pool = top.enter_context(tc.tile_pool(name="tk", bufs=1))
        gi_pool = top.enter_context(tc.tile_pool(name="gi", bufs=1))

        idf32 = const_pool.tile([128, 128], F32, tag="idf32")
        make_identity(nc, idf32[:])
        idbf = const_pool.tile([128, 128], BF16, tag="idbf")
        make_identity(nc, idbf[:])
        onescol = const_pool.tile([128, 1], F32, tag="ones")
        nc.vector.memset(onescol[:], 1.0)
        onesrow = const_pool.tile([1, 128], F32, tag="onesrow")
        nc.vector.memset(onesrow[:], 1.0)
        zeros512 = const_pool.tile([128, 512], F32, tag="zeros512")
        nc.vector.memset(zeros512[:], 0.0)

        def _zero_psum(ps_ap, lhsT_col):
            # start=True zeroes a whole 2KB PSUM bank region; open+close the
            # group per bank so banded accumulation can use start=False.
            ncols = ps_ap.shape[-1]
            for a, b in _split512(0, ncols):
                nc.tensor.matmul(
                    out=ps_ap[:, a:b], lhsT=lhsT_col, rhs=zeros512[:, : b - a],
                    start=True, stop=True, skip_group_check=True,
                )
        _bc_n = [0]

        def _bcast(ps_pool, out_ap, in_ap, P):
            # out[p, :] = in[0, :] via PE: ones[1,P].T @ in[1,N]
            N = in_ap.shape[-1]
            for a in range(0, N, 512):
                n = min(512, N - a)
                _bc_n[0] += 1
                ps = ps_pool.tile(
                    [P, 512], F32, tag="bcps", name=f"bcps{_bc_n[0]}"
                )
                nc.tensor.matmul(
                    out=ps[:, :n], lhsT=onesrow[:, :P], rhs=in_ap[:, a : a + n],
                    start=True, stop=True, skip_group_check=True,
                )
                nc.scalar.activation(
                    out=out_ap[:, a : a + n], in_=ps[:, :n], func=AF.Copy
                )

        myspanTb = myTb_pool.tile([128, SCH * KPC], BF16, tag="myspanTb")
        TTsb = TT_pool.tile([128, HCH * KPC], F32, tag="TTsb")
        topfast = tk_pool.tile([KPC, CP], F32, tag="topfast")
        topidx = tk_pool.tile([KPC, CP], U16, tag="topidx")

        with ExitStack() as mid:  # spanT & myspanT live through phase B only
            spanT_pool = mid.enter_context(tc.tile_pool(name="spanTp", bufs=1))
            myT_pool = mid.enter_context(tc.tile_pool(name="myT", bufs=1))
            # chunk ch columns [K*ch, K*ch+K) = span feature rows 128ch..+128
            spanT = spanT_pool.tile([128, SCH * K], F32, tag="spanT")
            myspanT = myT_pool.tile([128, SCH * KPC], F32, tag="myspanT")

            # ===== A1: start/end gathers, tables, transposes ==============
            with ExitStack() as ph:
                io_pool = ph.enter_context(tc.tile_pool(name="a1io", bufs=3))
                ps_pool = ph.enter_context(
                    tc.tile_pool(name="a1ps", bufs=4, space="PSUM")
                )
                for jb in range(DCH):
                    for src_idx, coloff in ((starts_p, 0), (ends_p, D)):
                        idxt = io_pool.tile([128, 1], I32, tag="gidx")
                        nc.sync.dma_start(
                            out=idxt[:],
                            in_=src_idx[128 * jb : 128 * jb + 128, None],
                        )
                        rows = io_pool.tile([128, D], F32, tag="grows")
                        nc.gpsimd.indirect_dma_start(
                            out=rows[:],
                            out_offset=None,
                            in_=md[:],
                            in_offset=bass.IndirectOffsetOnAxis(
                                ap=idxt[:, :1], axis=0
                            ),
                        )
                        rows_bf = io_pool.tile([128, D], BF16, tag="growsbf")
                        nc.vector.tensor_copy(out=rows_bf[:], in_=rows[:])
                        nc.sync.dma_start(
                            out=span_tab[
                                128 * jb : 128 * jb + 128, coloff : coloff + D
                            ],
                            in_=rows_bf[:],
                        )
                        nc.sync.dma_start(
                            out=span_f32[
                                128 * jb : 128 * jb + 128, coloff : coloff + D
                            ],
                            in_=rows[:],
                        )
                        for dc in range(DCH):
                            tp = ps_pool.tile([128, 128], F32, tag="tp")
                            nc.tensor.transpose(
                                out=tp[:],
                                in_=rows[:, 128 * dc : 128 * dc + 128],
                                identity=idf32[:],
                            )
                            ch = (coloff // 128) + dc
                            nc.scalar.activation(
                                out=spanT[
                                    :, K * ch + 128 * jb : K * ch + 128 * jb + 128
                                ],
                                in_=tp[:],
                                func=AF.Copy,
                            )

            # ===== A2a: hybrid sum -> DRAM, token attention ===============
            with ExitStack() as ph:
                ld_pool = ph.enter_context(tc.tile_pool(name="a2ld", bufs=3))
                sm_pool = ph.enter_context(tc.tile_pool(name="a2sm", bufs=1))
                wt_pool = ph.enter_context(tc.tile_pool(name="wt", bufs=1))

                bc_ps = ph.enter_context(
                    tc.tile_pool(name="bcps_a2", bufs=2, space="PSUM")
                )
                expv = sm_pool.tile([128, WCH], F32, tag="expv")
                attnraw = sm_pool.tile([128, WCH], F32, tag="attnraw")
                awhB = sm_pool.tile([128, D], F32, tag="awhB")
                awh_s = ld_pool.tile([1, D], F32, tag="awh1")
                nc.sync.dma_start(out=awh_s[:], in_=awh_p[:])
                _bcast(bc_ps, awhB[:], awh_s[:], 128)
                abv_s = sm_pool.tile([128, 1], F32, tag="abv")
                nc.sync.dma_start(out=abv_s[:], in_=abv_p[:])
                startsB = sm_pool.tile([128, K], F32, tag="startsB")
                endsB = sm_pool.tile([128, K], F32, tag="endsB")
                sf_s = ld_pool.tile([1, K], F32, tag="sf1")
                nc.sync.dma_start(out=sf_s[:], in_=startsf_p[:])
                _bcast(bc_ps, startsB[:], sf_s[:], 128)
                ef_s = ld_pool.tile([1, K], F32, tag="ef1")
                nc.sync.dma_start(out=ef_s[:], in_=endsf_p[:])
                _bcast(bc_ps, endsB[:], ef_s[:], 128)

                for t in range(WCH):
                    a = ld_pool.tile([128, D], F32, tag="mdt")
                    b = ld_pool.tile([128, D], F32, tag="md1t")
                    nc.sync.dma_start(out=a[:], in_=md[128 * t : 128 * t + 128, :])
                    nc.sync.dma_start(out=b[:], in_=md1[128 * t : 128 * t + 128, :])
                    hybt = ld_pool.tile([128, D], F32, tag="hybt")
                    nc.vector.tensor_add(out=hybt[:], in0=a[:], in1=b[:])
                    nc.sync.dma_start(
                        out=hyb_d[128 * t : 128 * t + 128, :], in_=hybt[:]
                    )
                    prod = ld_pool.tile([128, D], F32, tag="prod")
                    nc.vector.tensor_mul(out=prod[:], in0=hybt[:], in1=awhB[:])
                    nc.vector.tensor_reduce(
                        out=attnraw[:, t : t + 1], in_=prod[:], axis=AX, op=OP.add
                    )
                nc.scalar.activation(
                    out=expv[:], in_=attnraw[:], func=AF.Exp, bias=abv_s[:, :1]
                )

                # banded exp-weight matrix wT (kept for the head passes)
                wT = wt_pool.tile([128, int(wtoff[-1])], F32, tag="wT")
                widxi = sm_pool.tile([128, WCH], I32, tag="widxi")
                nc.gpsimd.iota(
                    widxi[:], pattern=[[128, WCH]], base=0, channel_multiplier=1
                )
                widx = sm_pool.tile([128, WCH], F32, tag="widx")
                nc.vector.tensor_copy(out=widx[:], in_=widxi[:])
                for t in range(WCH):
                    lo, hi = bandlo[t], bandhi[t]
                    if hi <= lo:
                        continue
                    sl = wT[:, int(wtoff[t]) : int(wtoff[t]) + (hi - lo)]
                    m1 = ld_pool.tile([128, MAXBW], F32, tag="m1")
                    nc.vector.tensor_scalar(
                        out=m1[:, : hi - lo], in0=startsB[:, lo:hi],
                        scalar1=widx[:, t : t + 1], scalar2=None, op0=OP.is_le,
                    )
                    nc.vector.tensor_scalar(
                        out=sl, in0=endsB[:, lo:hi], scalar1=widx[:, t : t + 1],
                        scalar2=expv[:, t : t + 1], op0=OP.is_ge, op1=OP.mult,
                    )
                    nc.vector.tensor_mul(out=sl, in0=sl, in1=m1[:, : hi - lo])

                # denominators + 0.5/denom broadcast
                rdenB = sm_pool.tile([128, K], F32, tag="rdenB")
                with ExitStack() as dph:
                    dps = dph.enter_context(
                        tc.tile_pool(name="dps", bufs=1, space="PSUM")
                    )
                    den_ps = dps.tile([1, K], F32, tag="denps")
                    _zero_psum(den_ps[:], onescol[:])
                    for t in range(WCH):
                        banded_matmuls(t, den_ps, onescol[:], wT)
                    rden = sm_pool.tile([1, K], F32, tag="rden")
                    nc.vector.reciprocal(out=rden[:], in_=den_ps[:])
                    nc.vector.tensor_scalar_mul(rden[:], rden[:], 0.5)
                    _bcast(bc_ps, rdenB[:], rden[:], 128)

                # ===== A2b: banded head matmuls, 2 passes x 3 d-chunks ====
                for dgrp in range(2):
                    with ExitStack() as hph:
                        hps = hph.enter_context(
                            tc.tile_pool(name=f"hps{dgrp}", bufs=1, space="PSUM")
                        )
                        hld = hph.enter_context(
                            tc.tile_pool(name=f"hld{dgrp}", bufs=3)
                        )
                        hpt = [
                            hps.tile([128, K], F32, tag=f"h{i}", name=f"hpt{i}")
                            for i in range(3)
                        ]
                        for i in range(3):
                            _zero_psum(hpt[i][:], idf32[:])
                        for t in range(WCH):
                            hybt = hld.tile([128, D], F32, tag="hybt2")
                            nc.sync.dma_start(
                                out=hybt[:], in_=hyb_d[128 * t : 128 * t + 128, :]
                            )
                            for i in range(3):
                                dc = 3 * dgrp + i
                                banded_matmuls(
                                    t, hpt[i],
                                    hybt[:, 128 * dc : 128 * dc + 128], wT,
                                )
                        for i in range(3):
                            ch = 12 + 3 * dgrp + i
                            nc.vector.tensor_mul(
                                out=spanT[:, K * ch : K * ch + K],
                                in0=hpt[i][:], in1=rdenB[:],
                            )

            # ===== A3: head rows -> tables ================================
            with ExitStack() as ph:
                hr_pool = ph.enter_context(tc.tile_pool(name="hrow", bufs=2))
                ps_pool = ph.enter_context(
                    tc.tile_pool(name="a3ps", bufs=4, space="PSUM")
                )
                for jb in range(DCH):
                    hrow = hr_pool.tile([128, D], F32, tag="hrow")
                    for dc in range(DCH):
                        ch = 12 + dc
                        tp = ps_pool.tile([128, 128], F32, tag="tp3")
                        nc.tensor.transpose(
                            out=tp[:],
                            in_=spanT[:, K * ch + 128 * jb : K * ch + 128 * jb + 128],
                            identity=idf32[:],
                        )
                        nc.scalar.activation(
                            out=hrow[:, 128 * dc : 128 * dc + 128],
                            in_=tp[:], func=AF.Copy,
                        )
                    hrow_bf = hr_pool.tile([128, D], BF16, tag="hrowbf")
                    nc.vector.tensor_copy(out=hrow_bf[:], in_=hrow[:])
                    nc.sync.dma_start(
                        out=span_tab[128 * jb : 128 * jb + 128, 2 * D : 3 * D],
                        in_=hrow_bf[:],
                    )
                    nc.sync.dma_start(
                        out=span_f32[128 * jb : 128 * jb + 128, 2 * D : 3 * D],
                        in_=hrow[:],
                    )

            # ===== B0: myspanT via indirect gather + transposes ===========
            with ExitStack() as ph:
                io_pool = ph.enter_context(tc.tile_pool(name="b0io", bufs=1))
                ps_pool = ph.enter_context(
                    tc.tile_pool(name="b0ps", bufs=4, space="PSUM")
                )
                myidx_s = io_pool.tile([128, 1], I32, tag="myidx")
                nc.sync.dma_start(out=myidx_s[:], in_=myidx_p[:])
                myrows = io_pool.tile([128, SPAN], F32, tag="myrows")
                nc.gpsimd.indirect_dma_start(
                    out=myrows[:],
                    out_offset=None,
                    in_=span_f32[:],
                    in_offset=bass.IndirectOffsetOnAxis(ap=myidx_s[:, :1], axis=0),
                )
                for ch in range(SCH):
                    tp = ps_pool.tile([128, KPC], F32, tag="tpb")
                    nc.tensor.transpose(
                        out=tp[:],
                        in_=myrows[:KPC, 128 * ch : 128 * ch + 128],
                        identity=idf32[:KPC, :KPC],
                    )
                    nc.scalar.activation(
                        out=myspanT[:, KPC * ch : KPC * ch + KPC],
                        in_=tp[:], func=AF.Copy,
                    )
                    nc.vector.tensor_copy(
                        out=myspanTb[:, KPC * ch : KPC * ch + KPC],
                        in_=myspanT[:, KPC * ch : KPC * ch + KPC],
                    )

            # ===== B1: A table (span @ W1a, bf16) =========================
            with ExitStack() as ph:
                w_pool = ph.enter_context(tc.tile_pool(name="b1w", bufs=1))
                sb_pool = ph.enter_context(tc.tile_pool(name="b1sb", bufs=2))
                ps_pool = ph.enter_context(
                    tc.tile_pool(name="b1ps", bufs=2, space="PSUM")
                )
                w1a_s = w_pool.tile([128, SCH * HP], BF16, tag="w1a")
                for ch in range(SCH):
                    nc.sync.dma_start(
                        out=w1a_s[:, HP * ch : HP * ch + HP],
                        in_=w1a_p[128 * ch : 128 * ch + 128, :],
                    )
                spanTb = w_pool.tile([128, SCH * K], BF16, tag="spanTb")
                for ch in range(SCH):
                    nc.vector.tensor_copy(
                        out=spanTb[:, K * ch : K * ch + K],
                        in_=spanT[:, K * ch : K * ch + K],
                    )
                for jb in range(DCH):
                    aps = ps_pool.tile([128, HP], F32, tag="aps")
                    for ch in range(SCH):
                        for nh in range(2):
                            nc.tensor.matmul(
                                out=aps[:, 512 * nh : 512 * nh + 512],
                                lhsT=spanTb[
                                    :, K * ch + 128 * jb : K * ch + 128 * jb + 128
                                ],
                                rhs=w1a_s[
                                    :, HP * ch + 512 * nh : HP * ch + 512 * nh + 512
                                ],
                                start=(ch == 0), stop=(ch == SCH - 1),
                                skip_group_check=True,
                            )
                    abf = sb_pool.tile([128, HP], BF16, tag="abf")
                    nc.scalar.activation(out=abf[:], in_=aps[:], func=AF.Copy)
                    nc.sync.dma_start(
                        out=a_tab[128 * jb : 128 * jb + 128, :], in_=abf[:]
                    )

            # ===== B2: TT = W1t^T @ myspanT (bf16) ========================
            with ExitStack() as ph:
                w_pool = ph.enter_context(tc.tile_pool(name="b2w", bufs=1))
                ps_pool = ph.enter_context(
                    tc.tile_pool(name="b2ps", bufs=1, space="PSUM")
                )
                w1t_s = w_pool.tile([128, SCH * HP], BF16, tag="w1t")
                for ch in range(SCH):
                    nc.sync.dma_start(
                        out=w1t_s[:, HP * ch : HP * ch + HP],
                        in_=w1t_p[128 * ch : 128 * ch + 128, :],
                    )
                ttps = ps_pool.tile([128, HCH * 128], F32, tag="ttps")
                for s in range(HCH):
                    for ch in range(SCH):
                        nc.tensor.matmul(
                            out=ttps[:, 128 * s : 128 * s + KPC],
                            lhsT=w1t_s[
                                :, HP * ch + 128 * s : HP * ch + 128 * s + 128
                            ],
                            rhs=myspanTb[:, KPC * ch : KPC * ch + KPC],
                            start=(ch == 0), stop=(ch == SCH - 1),
                            skip_group_check=True,
                        )
                nc.vector.tensor_copy(
                    out=TTsb[:].rearrange("p (s k) -> p s k", k=KPC),
                    in_=ttps[:].rearrange("p (s k) -> p s k", k=128)[:, :, :KPC],
                )

            # ===== B3: srcT (coarse) then fast + topk =====================
            with ExitStack() as ph:
                cw_pool = ph.enter_context(tc.tile_pool(name="cwp", bufs=3))
                src_pool = ph.enter_context(tc.tile_pool(name="srcsb", bufs=1))
                ps_pool = ph.enter_context(
                    tc.tile_pool(name="b3ps", bufs=1, space="PSUM")
                )
                srcT = src_pool.tile([128, SCH * KPC], F32, tag="srcT")
                cbt_s = src_pool.tile([128, SCH], F32, tag="cbt")
                nc.sync.dma_start(out=cbt_s[:], in_=cbt_p[:])
                sps = ps_pool.tile([128, SCH * 128], F32, tag="sps")
                _zero_psum(sps[:], idf32[:])
                for dch in range(SCH):
                    cwt = cw_pool.tile([128, SPAN], F32, tag="cwt")
                    nc.sync.dma_start(
                        out=cwt[:], in_=cw_p[128 * dch : 128 * dch + 128, :]
                    )
                    for oc in range(SCH):
                        nc.tensor.matmul(
                            out=sps[:, 128 * oc : 128 * oc + KPC],
                            lhsT=cwt[:, 128 * oc : 128 * oc + 128],
                            rhs=myspanT[:, KPC * dch : KPC * dch + KPC],
                            start=False, stop=False,
                            skip_group_check=True,
                        )
                for oc in range(SCH):
                    nc.vector.tensor_scalar(
                        out=srcT[:, KPC * oc : KPC * oc + KPC],
                        in0=sps[:, 128 * oc : 128 * oc + KPC],
                        scalar1=cbt_s[:, oc : oc + 1], scalar2=None, op0=OP.add,
                    )
                with ExitStack() as fph:
                    f_pool = fph.enter_context(tc.tile_pool(name="fast", bufs=1))
                    fps_pool = fph.enter_context(
                        tc.tile_pool(name="fps", bufs=1, space="PSUM")
                    )
                    fps = fps_pool.tile([KPC, K], F32, tag="fps")
                    for ch in range(SCH):
                        for a, b in ((0, 512), (512, K)):
                            nc.tensor.matmul(
                                out=fps[:, a:b],
                                lhsT=srcT[:, KPC * ch : KPC * ch + KPC],
                                rhs=spanT[:, K * ch + a : K * ch + b],
                                start=(ch == 0), stop=(ch == SCH - 1),
                                skip_group_check=True,
                            )
                    bias_s = f_pool.tile([KPC, K], F32, tag="biasmx")
                    nc.sync.dma_start(out=bias_s[:], in_=biasmx_p[:])
                    fast = f_pool.tile([KPC, K], F32, tag="fastsb")
                    nc.vector.tensor_add(out=fast[:], in0=fps[:], in1=bias_s[:])
                    for r in range(7):
                        vs = topfast[:, 8 * r : 8 * r + 8]
                        nc.vector.max(out=vs, in_=fast[:])
                        nc.vector.max_index(
                            out=topidx[:, 8 * r : 8 * r + 8],
                            in_max=vs, in_values=fast[:],
                        )
                        nc.vector.match_replace(
                            out=fast[:], in_to_replace=vs, in_values=fast[:],
                            imm_value=NEG,
                        )
        # ---- spanT / myspanT freed here ----------------------------------

        # ===== B4: gather indices + bucket one-hot ========================
        idxall = gi_pool.tile([128, NPAIR // 16], I16, tag="idxall")
        onehot = gi_pool.tile([10, NPAIR], BF16, tag="onehot")
        with ExitStack() as ph:
            w_pool = ph.enter_context(tc.tile_pool(name="b4", bufs=1))
            nc.sync.dma_start(
                out=idxflat_d[:].rearrange("(k c) -> k c", c=CP), in_=topidx[:]
            )
            nc.sync.dma_start(
                out=idxall[:16, :].bitcast(U16),
                in_=idxflat_d[:].rearrange("(s p) -> p s", p=16),
            )
            for g in range(1, 8):
                nc.sync.dma_start(
                    out=idxall[16 * g : 16 * g + 16, :], in_=idxall[:16, :]
                )
            kvec_s = w_pool.tile([KPC, 1], F32, tag="kvec")
            nc.sync.dma_start(out=kvec_s[:], in_=kvec_p[:])
            idxf = w_pool.tile([KPC, CP], F32, tag="idxf")
            nc.vector.tensor_copy(out=idxf[:], in_=topidx[:])
            dmat = w_pool.tile([KPC, CP], F32, tag="dmat")
            nc.vector.tensor_scalar(
                out=dmat[:], in0=idxf[:], scalar1=-1.0, scalar2=kvec_s[:, :1],
                op0=OP.mult, op1=OP.add,
            )
            bkt = w_pool.tile([KPC, CP], F32, tag="bkt")
            nc.vector.tensor_scalar(
                out=bkt[:], in0=dmat[:], scalar1=4.0, scalar2=None, op0=OP.min
            )
            tmp = w_pool.tile([KPC, CP], F32, tag="btmp")
            for tau in (5.0, 8.0, 16.0, 32.0, 64.0):
                nc.vector.tensor_scalar(
                    out=tmp[:], in0=dmat[:], scalar1=tau, scalar2=None,
                    op0=OP.is_ge,
                )
                nc.vector.tensor_add(out=bkt[:], in0=bkt[:], in1=tmp[:])
            nc.sync.dma_start(
                out=bucket_d[:].rearrange("(k c) -> k c", c=CP), in_=bkt[:]
            )
            brow = w_pool.tile([1, NPAIR], F32, tag="brow")
            nc.sync.dma_start(out=brow[:], in_=bucket_d[None, :])
            bktB = w_pool.tile([10, NPAIR], F32, tag="bktB")
            with tc.tile_pool(name="bcps_b4", bufs=2, space="PSUM") as bc_ps4:
                _bcast(bc_ps4, bktB[:], brow[:], 10)
            ivi = w_pool.tile([10, 1], I32, tag="ivi")
            nc.gpsimd.iota(ivi[:], pattern=[[0, 1]], base=0, channel_multiplier=1)
            ivf = w_pool.tile([10, 1], F32, tag="ivf")
            nc.vector.tensor_copy(out=ivf[:], in_=ivi[:])
            nc.vector.tensor_scalar(
                out=onehot[:], in0=bktB[:], scalar1=ivf[:, :1], scalar2=None,
                op0=OP.is_equal,
            )

        # ===== C: gather + FFNN pipeline ==================================
        # shared count registers: each dma_gather otherwise permanently
        # allocates a fresh gpsimd register and exhausts the pool
        nreg = {n: nc.gpsimd.to_reg(n) for n in (128, 512, 384)}
        with ExitStack() as ph:
            w_pool = ph.enter_context(tc.tile_pool(name="cw1s", bufs=1))
            st_pool = ph.enter_context(tc.tile_pool(name="cst", bufs=1))
            pa_pool = ph.enter_context(tc.tile_pool(name="cpa", bufs=2))
            at_pool = ph.enter_context(tc.tile_pool(name="cat", bufs=1))
            r_pool = ph.enter_context(tc.tile_pool(name="crelu", bufs=2))
            sf_pool = ph.enter_context(tc.tile_pool(name="csf", bufs=2))
            ps_pool = ph.enter_context(
                tc.tile_pool(name="cps", bufs=2, space="PSUM")
            )
            sp_pool = ph.enter_context(
                tc.tile_pool(name="cslow", bufs=1, space="PSUM")
            )
            w1s_s = w_pool.tile([128, SCH * HP], BF16, tag="w1s")
            for ch in range(SCH):
                nc.sync.dma_start(
                    out=w1s_s[:, HP * ch : HP * ch + HP],
                    in_=w1s_p[128 * ch : 128 * ch + 128, :],
                )
            d10_s = w_pool.tile([16, HP], BF16, tag="d10")
            nc.sync.dma_start(out=d10_s[:], in_=d10_p[:])
            w2t_s = w_pool.tile([128, HCH], BF16, tag="w2t")
            nc.sync.dma_start(out=w2t_s[:], in_=w2t_p[:])
            b1t_s = w_pool.tile([128, HCH], F32, tag="b1t")
            nc.sync.dma_start(out=b1t_s[:], in_=b1t_p[:])

            for b in range(NBLK):
                p0 = BLK * b
                stile = st_pool.tile([128, NSUB * SPAN], BF16, tag="stile")
                for c in range(NSUB):
                    oap = stile[:, SPAN * c : SPAN * c + SPAN].rearrange(
                        "p (ch f) -> p ch f", f=128
                    )
                    nc.gpsimd.dma_gather(
                        out_ap=oap,
                        in_ap=span_tab[:],
                        idxs_ap=idxall[
                            :, (p0 + 128 * c) // 16 : (p0 + 128 * c) // 16 + 8
                        ],
                        num_idxs=128,
                        num_idxs_reg=nreg[128],
                        elem_size=SPAN,
                        transpose=True,
                        queue_num=0,
                    )
                atile0 = at_pool.tile([128, 8 * 512], BF16, tag="atile0")
                atile1 = at_pool.tile([128, 8 * 384], BF16, tag="atile1")
                for atile, off, n in ((atile0, 0, 512), (atile1, 512, 384)):
                    nc.gpsimd.dma_gather(
                        out_ap=atile[:].rearrange("p (ch f) -> p ch f", f=n),
                        in_ap=a_tab[:],
                        idxs_ap=idxall[:, (p0 + off) // 16 : (p0 + off + n) // 16],
                        num_idxs=n,
                        num_idxs_reg=nreg[n],
                        elem_size=HP,
                        transpose=True,
                        queue_num=0,
                    )
                pairt = pa_pool.tile([128, SCH * BLK], BF16, tag="pairt")
                sview = stile[:].rearrange(
                    "p (c ch f) -> p ch c f", c=NSUB, ch=SCH
                )
                for ch in range(SCH):
                    dst = pairt[:, BLK * ch : BLK * ch + BLK]
                    nc.vector.tensor_copy(out=dst, in_=sview[:, ch])
                    nc.vector.tensor_tensor(
                        out=dst.rearrange("p (g c) -> p g c", c=CP),
                        in0=dst.rearrange("p (g c) -> p g c", c=CP),
                        in1=myspanTb[
                            :, KPC * ch + KGRP * b : KPC * ch + KGRP * b + KGRP
                        ][:, :, None].to_broadcast([128, KGRP, CP]),
                        op=OP.mult,
                    )
                sps2 = sp_pool.tile([1, BLK], F32, tag="slowps")
                for s in range(HCH):
                    pps = ps_pool.tile([128, BLK], F32, tag="pps")
                    for ch in range(SCH):
                        for o2, n2 in ((0, 512), (512, BLK - 512)):
                            nc.tensor.matmul(
                                out=pps[:, o2 : o2 + n2],
                                lhsT=w1s_s[
                                    :, HP * ch + 128 * s : HP * ch + 128 * s + 128
                                ],
                                rhs=pairt[:, BLK * ch + o2 : BLK * ch + o2 + n2],
                                start=(ch == 0), stop=False,
                                skip_group_check=True,
                            )
                    nc.tensor.matmul(
                        out=pps[:, 0:512], lhsT=idbf[:],
                        rhs=atile0[:, 512 * s : 512 * s + 512],
                        start=False, stop=False, skip_group_check=True,
                    )
                    nc.tensor.matmul(
                        out=pps[:, 512:BLK], lhsT=idbf[:],
                        rhs=atile1[:, 384 * s : 384 * s + 384],
                        start=False, stop=False, skip_group_check=True,
                    )
                    for o2, n2 in ((0, 512), (512, BLK - 512)):
                        nc.tensor.matmul(
                            out=pps[:, o2 : o2 + n2],
                            lhsT=d10_s[:10, 128 * s : 128 * s + 128],
                            rhs=onehot[:, p0 + o2 : p0 + o2 + n2],
                            start=False, stop=True, skip_group_check=True,
                        )
                    nc.vector.tensor_tensor(
                        out=pps[:].rearrange("p (g c) -> p g c", c=CP),
                        in0=pps[:].rearrange("p (g c) -> p g c", c=CP),
                        in1=TTsb[
                            :, KPC * s + KGRP * b : KPC * s + KGRP * b + KGRP
                        ][:, :, None].to_broadcast([128, KGRP, CP]),
                        op=OP.add,
                    )
                    relu_bf = r_pool.tile([128, BLK], BF16, tag="relubf")
                    nc.scalar.activation(
                        out=relu_bf[:], in_=pps[:], func=AF.Relu,
                        bias=b1t_s[:, s : s + 1],
                    )
                    for o2, n2 in ((0, 512), (512, BLK - 512)):
                        nc.tensor.matmul(
                            out=sps2[:, o2 : o2 + n2],
                            lhsT=w2t_s[:, s : s + 1],
                            rhs=relu_bf[:, o2 : o2 + n2],
                            start=(s == 0), stop=(s == HCH - 1),
                            skip_group_check=True,
                        )
                sflat = sf_pool.tile([1, BLK], F32, tag="sflat")
                nc.scalar.activation(out=sflat[:], in_=sps2[:], func=AF.Copy)
                nc.sync.dma_start(out=slow_d[None, p0 : p0 + BLK], in_=sflat[:])

        # ===== D: assembly ================================================
        with ExitStack() as ph:
            w_pool = ph.enter_context(tc.tile_pool(name="fin", bufs=1))
            slow56 = w_pool.tile([KPC, CP], F32, tag="slow56")
            nc.sync.dma_start(
                out=slow56[:], in_=slow_d[:].rearrange("(k c) -> k c", c=CP)
            )
            b2v_s = w_pool.tile([KPC, 1], F32, tag="b2v")
            nc.sync.dma_start(out=b2v_s[:], in_=b2v_p[:])
            forcem_s = w_pool.tile([KPC, 51], F32, tag="forcem")
            nc.sync.dma_start(out=forcem_s[:], in_=forcem_p[:])
            t1 = w_pool.tile([KPC, CP], F32, tag="t1")
            nc.vector.tensor_add(out=t1[:], in0=slow56[:], in1=topfast[:])
            out96 = w_pool.tile([KPC, 51], F32, tag="out96")
            nc.vector.memset(out96[:], 0.0)
            nc.vector.tensor_scalar(
                out=out96[:, 1:51], in0=t1[:, 0:C], scalar1=b2v_s[:, :1],
                scalar2=None, op0=OP.add,
            )
            nc.vector.tensor_add(out=out96[:], in0=out96[:], in1=forcem_s[:])
            nc.sync.dma_start(out=out_p[:], in_=out96[:])

    return nc


def prepare(inputs):
    gi = {k: np.asarray(v) for k, v in inputs.items()}
    md = gi["mention_doc"].astype(np.float32)
    md1 = gi["mention_doc_one"].astype(np.float32)
    starts = gi["top_span_starts"].astype(np.int64)
    ends = gi["top_span_ends"].astype(np.int64)
    ms = gi["top_span_mention_scores"].astype(np.float32)
    attn_w = gi["attn_w"].astype(np.float32)
    attn_b = np.float32(gi["attn_b"])
    coarse_w = gi["coarse_w"].astype(np.float32)
    coarse_b = gi["coarse_b"].astype(np.float32)
    dist_prior_emb = gi["dist_prior_emb"].astype(np.float32)
    dist_w = gi["dist_w"].astype(np.float32)
    dist_b = np.float32(gi["dist_b"])
    top_dist_emb = gi["top_dist_emb"].astype(np.float32)
    w1 = gi["coref_w1"].astype(np.float32)
    b1 = gi["coref_b1"].astype(np.float32)
    w2 = gi["coref_w2"].astype(np.float32)
    b2 = np.float32(gi["coref_b2"])

    PAIR = 3 * SPAN + F
    assert w1.shape == (PAIR, H)
    w1_tgt = w1[0:SPAN]
    w1_ant = w1[SPAN : 2 * SPAN]
    w1_sim = w1[2 * SPAN : 3 * SPAN]
    w1_dst = w1[3 * SPAN :]

    def padh(x):
        return np.concatenate(
            [x, np.zeros((*x.shape[:-1], HP - H), x.dtype)], axis=-1
        )

    w1t_bf = np.ascontiguousarray(padh(w1_tgt).astype(BF))
    w1a_bf = np.ascontiguousarray(padh(w1_ant).astype(BF))
    w1s_bf = np.ascontiguousarray(padh(w1_sim).astype(BF))
    d10 = top_dist_emb @ w1_dst
    d10_bf = np.zeros((16, HP), dtype=BF)
    d10_bf[:10] = padh(d10).astype(BF)
    b1t = np.ascontiguousarray(padh(b1[None, :]).reshape(HCH, 128).T).astype(
        np.float32
    )
    w2t = np.ascontiguousarray(padh(w2[None, :]).reshape(HCH, 128).T.astype(BF))
    cbt = np.ascontiguousarray(coarse_b.reshape(SCH, 128).T).astype(np.float32)
    dist_score = (dist_prior_emb @ dist_w + dist_b).astype(np.float32)

    tok = np.arange(K)
    in_maps = []
    for m in range(NCORES):
        rb = m * KPC
        krange = np.arange(rb, rb + KPC)
        dmat = krange[:, None] - tok[None, :]
        bias = ms[krange][:, None] + ms[None, :] + dist_score[_bucket_np(dmat)]
        bias = np.where(dmat >= 1, bias, NEG).astype(np.float32)
        forcem = np.zeros((KPC, 51), np.float32)
        cc = np.arange(50)[None, :]
        forcem[:, 1:51] = np.where(cc >= krange[:, None], -np.inf, 0.0)
        myidx = np.zeros((128, 1), np.int32)
        myidx[:KPC, 0] = krange
        in_maps.append(
            {
                "md": md,
                "md1": md1,
                "starts": starts.astype(np.int32),
                "ends": ends.astype(np.int32),
                "startsf": np.ascontiguousarray(
                    starts.astype(np.float32)[None, :]
                ),
                "endsf": np.ascontiguousarray(ends.astype(np.float32)[None, :]),
                "awh": np.ascontiguousarray((attn_w * 0.5)[None, :]),
                "abv": np.full((128, 1), attn_b, np.float32),
                "cw": coarse_w,
                "cbt": cbt,
                "w1t": w1t_bf,
                "w1a": w1a_bf,
                "w1s": w1s_bf,
                "d10": d10_bf,
                "b1t": b1t,
                "w2t": w2t,
                "b2v": np.full((KPC, 1), b2, np.float32),
                "biasmx": bias,
                "forcem": forcem,
                "myidx": myidx,
                "kvec": np.ascontiguousarray(
                    krange[:, None].astype(np.float32)
                ),
            }
        )

    nc = build_program(starts)

    def assemble(results):
        out = np.zeros((K, 51), np.float32)
        for m in range(NCORES):
            out[m * KPC : (m + 1) * KPC] = results[m]["out"]
        return out

    return nc, in_maps, assemble


def kernel(**inputs):
    nc, in_maps, assemble = prepare(inputs)
    res = run_bass_kernel_spmd(nc, in_maps, list(range(NCORES)))
    return assemble(res.results)
